# revision 4
# baseline (speedup 1.0000x reference)
"""Trainium2 Bass kernel for the chunked-scan final-state problem.

Math: the reference's chunked scan + inter-chunk segsum reduces exactly to
    out[b, h, p, n] = sum_t exp(sum_{t'>t} A[b, t', h]) * X[b, t, h, p] * B[b, t, h, n]
(input C is unused by the reference).  Per (b, h) this is a (64 x 2048) @
(2048 x 16) matmul with a decay weight folded into B.

Distribution: data-parallel over batch, 8 batches per core, 8 cores.

Per-core kernel (NB=8 batches):
  - A[b] is loaded as (128 q x (i, h)) where t = 128*i + q (16 K-tiles).
  - PE computes, per b, two matmuls against constant 128x128 matrices:
      P1 = Lstrict^T @ A1   (strict suffix-sum of A within each 128-tile)
      P2 = ones^T  @ A1     (tile totals broadcast to all partitions)
  - A 4-step Hillis-Steele pass over the 16 tile-groups (free dim) turns P2
    into the strict suffix of tile totals; w = exp(P1 + carry).
  - B[b] tiles are weighted by w (free-dim broadcast over n).
  - Main matmuls: stationary = X[b] 2-head slice (128 x 128), moving =
    weighted-B 2-head slice (128 x 32), accumulated over 16 K-tiles into
    PSUM; the two diagonal (64 x 16) blocks are the per-head outputs in
    (p, n) layout, DMA'd straight to DRAM.
"""

import numpy as np

import concourse.bacc as bacc
import concourse.mybir as mybir
import concourse.tile as tile
from concourse.bass_utils import run_bass_kernel_spmd
from concourse.masks import make_lower_triangular

F32 = mybir.dt.float32
NCORES = 8
NB = 8          # batches per core
T = 2048        # sequence length
NH = 8          # heads
DP = 64         # d_head
DN = 16         # d_state
NT = T // 128   # K-tiles of 128


_NC_CACHE = None


def _build():
    global _NC_CACHE
    if _NC_CACHE is not None:
        return _NC_CACHE

    nc = bacc.Bacc("TRN2", target_bir_lowering=False, debug=False)
    Xd = nc.dram_tensor("X", (NB, T, NH, DP), F32, kind="ExternalInput").ap()
    Ad = nc.dram_tensor("A", (NB, T, NH), F32, kind="ExternalInput").ap()
    Bd = nc.dram_tensor("B", (NB, T, NH, DN), F32, kind="ExternalInput").ap()
    Od = nc.dram_tensor("O", (NB, NH, DP, DN), F32, kind="ExternalOutput").ap()

    with tile.TileContext(nc) as tc:
        with (
            tc.tile_pool(name="consts", bufs=1) as cpool,
            tc.tile_pool(name="a1p", bufs=2) as apool,
            tc.tile_pool(name="scan", bufs=1) as spool,
            tc.tile_pool(name="wexp", bufs=2) as wpool,
            tc.tile_pool(name="bmat", bufs=2) as bpool,
            tc.tile_pool(name="bwp", bufs=2) as bwpool,
            tc.tile_pool(name="xmat", bufs=2) as xpool,
            tc.tile_pool(name="outs", bufs=4) as opool,
            tc.tile_pool(name="ps_scan", bufs=2, space="PSUM") as pspool,
            tc.tile_pool(name="ps_main", bufs=4, space="PSUM") as pmpool,
        ):
            # constants: all-ones and strict lower-triangular (L[k, m] = 1 iff k > m)
            ones = cpool.tile([128, 128], F32)
            nc.gpsimd.memset(ones[:], 1.0)
            ltri = cpool.tile([128, 128], F32)
            make_lower_triangular(nc, ltri[:], val=1.0, diag=False)

            # scan ping-pong buffers; cols >=120/128 stay zero forever
            va = spool.tile([128, 192], F32, tag="va")
            vb = spool.tile([128, 192], F32, tag="vb")
            nc.vector.memset(va[:, 120:192], 0.0)
            nc.vector.memset(vb[:, 128:192], 0.0)

            for b in range(NB):
                # ---- decay weights w[q, (i,h)] = exp(suffix-sum of A after t) ----
                a1 = apool.tile([128, 128], F32)
                nc.sync.dma_start(
                    out=a1[:].rearrange("q (i h) -> q i h", i=NT),
                    in_=Ad[b].rearrange("(i q) h -> q i h", q=128),
                )
                p2 = pspool.tile([128, 128], F32, tag="p2")
                nc.tensor.matmul(p2[:], ones[:], a1[:], start=True, stop=True)
                p1 = pspool.tile([128, 128], F32, tag="p1")
                nc.tensor.matmul(p1[:], ltri[:], a1[:], start=True, stop=True)

                # strict suffix over the 16 tile-groups (group = 8 cols)
                nc.vector.tensor_copy(va[:, 0:120], p2[:, 8:128])
                nc.vector.tensor_add(vb[:, 0:128], va[:, 0:128], va[:, 8:136])
                nc.vector.tensor_add(va[:, 0:128], vb[:, 0:128], vb[:, 16:144])
                nc.vector.tensor_add(vb[:, 0:128], va[:, 0:128], va[:, 32:160])
                nc.vector.tensor_add(va[:, 0:128], vb[:, 0:128], vb[:, 64:192])

                wpre = wpool.tile([128, 128], F32, tag="wpre")
                nc.vector.tensor_add(wpre[:], p1[:], va[:, 0:128])
                w = wpool.tile([128, 128], F32, tag="w")
                nc.scalar.activation(w[:], wpre[:], mybir.ActivationFunctionType.Exp)

                # ---- B load + decay weighting (broadcast w over n) ----
                bt = bpool.tile([128, NT * 128], F32)
                nc.sync.dma_start(
                    out=bt[:].rearrange("q (i h n) -> q i h n", i=NT, h=NH),
                    in_=Bd[b].rearrange("(i q) h n -> q i h n", q=128),
                )
                bw = bwpool.tile([128, NT * 128], F32)
                nc.vector.tensor_mul(
                    bw[:].rearrange("q (ih n) -> q ih n", n=DN),
                    bt[:].rearrange("q (ih n) -> q ih n", n=DN),
                    w[:].unsqueeze(2).broadcast_to((128, 128, DN)),
                )

                # ---- X load (contiguous 4 MB) ----
                xt = xpool.tile([128, NT * 512], F32)
                nc.sync.dma_start(
                    out=xt[:].rearrange("q (i h p) -> q i h p", i=NT, h=NH),
                    in_=Xd[b].rearrange("(i q) h p -> q i h p", q=128),
                )

                # ---- main matmuls: 4 head-pair groups x 16 K-tiles ----
                for g in range(4):
                    pm = pmpool.tile([128, 32], F32, tag="pm")
                    for i in range(NT):
                        nc.tensor.matmul(
                            pm[:],
                            xt[:, i * 512 + g * 128 : i * 512 + (g + 1) * 128],
                            bw[:, i * 128 + g * 32 : i * 128 + (g + 1) * 32],
                            start=(i == 0),
                            stop=(i == NT - 1),
                        )
                    ot = opool.tile([128, 32], F32)
                    nc.vector.tensor_copy(ot[:], pm[:])
                    # diagonal blocks are the per-head (p, n) outputs
                    nc.scalar.dma_start(out=Od[b, 2 * g], in_=ot[0:64, 0:16])
                    nc.scalar.dma_start(out=Od[b, 2 * g + 1], in_=ot[64:128, 16:32])

    nc.compile()
    _NC_CACHE = nc
    return nc


def run(inputs, trace=False, tmpdir=None, trace_kwargs=None):
    """Run the SPMD kernel on 8 cores.  Returns (output, BassKernelResults)."""
    X = np.asarray(inputs["X"], dtype=np.float32)
    A = np.asarray(inputs["A"], dtype=np.float32)
    B = np.asarray(inputs["B"], dtype=np.float32)
    assert X.shape == (NCORES * NB, T, NH, DP), X.shape

    nc = _build()
    in_maps = []
    for c in range(NCORES):
        s = slice(c * NB, (c + 1) * NB)
        in_maps.append(
            {
                "X": np.ascontiguousarray(X[s]),
                "A": np.ascontiguousarray(A[s]),
                "B": np.ascontiguousarray(B[s]),
            }
        )
    kw = {}
    if trace:
        kw.update(trace=True, tmpdir=tmpdir, trace_kwargs=trace_kwargs or {})
    res = run_bass_kernel_spmd(nc, in_maps, core_ids=list(range(NCORES)), **kw)
    out = np.concatenate([res.results[c]["O"] for c in range(NCORES)], axis=0)
    return out, res


def kernel(**inputs) -> np.ndarray:
    out, _ = run(inputs)
    return out


# revision 5
# speedup vs baseline: 1.5080x; 1.5080x over previous
"""Trainium2 Bass kernel for the chunked-scan final-state problem.

Math: the reference's chunked scan + inter-chunk segsum reduces exactly to
    out[b, h, p, n] = sum_t exp(sum_{t'>t} A[b, t', h]) * X[b, t, h, p] * B[b, t, h, n]
(input C is unused by the reference).  Per (b, h) this is a (64 x 2048) @
(2048 x 16) matmul with a decay weight folded into B.

Distribution: data-parallel over batch, 8 batches per core, 8 cores.

Layout trick ("comb" K-tiles): contraction tile i takes t in {16q + i},
q = partition.  Then every DMA is fully contiguous (partition q reads rows
16q..16q+15: X 32KB, B 8KB, A 512B runs) and the decay suffix-sum becomes
  w[q, (i,h)] = exp( suffix_i(A_row q) + carry[q, h] )
where suffix_i is a 4-step shifted-add scan along the free dim and
carry = Lstrict^T @ row_totals is one small PE matmul over partitions.

Main matmuls (per batch, 16 K-tiles): stationary = weighted-B tile
(128 x 128 = all 8 heads), moving = X tile (128 x 512) -> PSUM (128 x 512)
accumulated over i; entry ((h'n), (h''p)).  The diagonal h'=h'' blocks are
the per-head outputs in (n, p) orientation; a DVE 32x32 blockwise
transpose + strided DMA writes them as (p, n) to DRAM.
"""

import numpy as np

import concourse.bacc as bacc
import concourse.mybir as mybir
import concourse.tile as tile
from concourse.bass_utils import run_bass_kernel_spmd
from concourse.masks import make_lower_triangular

F32 = mybir.dt.float32
NCORES = 8
NB = 8          # batches per core
T = 2048        # sequence length
NH = 8          # heads
DP = 64         # d_head
DN = 16         # d_state
NT = T // 128   # K-tiles of 128

_NC_CACHE = None


def _build():
    global _NC_CACHE
    if _NC_CACHE is not None:
        return _NC_CACHE

    nc = bacc.Bacc("TRN2", target_bir_lowering=False, debug=False)
    Xd = nc.dram_tensor("X", (NB, T, NH, DP), F32, kind="ExternalInput").ap()
    Ad = nc.dram_tensor("A", (NB, T, NH), F32, kind="ExternalInput").ap()
    Bd = nc.dram_tensor("B", (NB, T, NH, DN), F32, kind="ExternalInput").ap()
    Od = nc.dram_tensor("O", (NB, NH, DP, DN), F32, kind="ExternalOutput").ap()

    with tile.TileContext(nc) as tc:
        with (
            tc.tile_pool(name="consts", bufs=1) as cpool,
            tc.tile_pool(name="a1p", bufs=2) as apool,
            tc.tile_pool(name="scan", bufs=1) as spool,
            tc.tile_pool(name="wexp", bufs=2) as wpool,
            tc.tile_pool(name="bmat", bufs=2) as bpool,
            tc.tile_pool(name="bwp", bufs=2) as bwpool,
            tc.tile_pool(name="xmat", bufs=2) as xpool,
            tc.tile_pool(name="outs", bufs=3) as opool,
            tc.tile_pool(name="ps_carry", bufs=2, space="PSUM") as pcpool,
            tc.tile_pool(name="ps_main", bufs=3, space="PSUM") as pmpool,
        ):
            # strict lower-triangular constant: L[k, m] = 1 iff k > m
            ltri = cpool.tile([128, 128], F32)
            make_lower_triangular(nc, ltri[:], val=1.0, diag=False)

            # scan ping-pong buffers; pad cols stay zero forever
            va = spool.tile([128, 192], F32, tag="va")
            vb = spool.tile([128, 192], F32, tag="vb")
            nc.vector.memset(va[:, 120:192], 0.0)
            nc.vector.memset(vb[:, 128:192], 0.0)

            for b in range(NB):
                # ---- A load (fully contiguous 64 KB) ----
                a1 = apool.tile([128, 128], F32)
                nc.sync.dma_start(
                    out=a1[:].rearrange("q (i h) -> q i h", i=NT),
                    in_=Ad[b].rearrange("(q i) h -> q i h", q=128),
                )

                # ---- strict suffix over i (16 groups of 8 cols) ----
                nc.vector.tensor_copy(va[:, 0:120], a1[:, 8:128])
                nc.vector.tensor_add(vb[:, 0:128], va[:, 0:128], va[:, 8:136])
                nc.vector.tensor_add(va[:, 0:128], vb[:, 0:128], vb[:, 16:144])
                nc.vector.tensor_add(vb[:, 0:128], va[:, 0:128], va[:, 32:160])
                nc.vector.tensor_add(va[:, 0:128], vb[:, 0:128], vb[:, 64:192])

                # row totals T[q, h] = strict_suffix(i=0) + A(i=0)
                tt = wpool.tile([128, 8], F32, tag="tt")
                nc.vector.tensor_add(tt[:], va[:, 0:8], a1[:, 0:8])
                # carry[q, h] = sum_{q' > q} T[q', h]  (partition-dim suffix)
                pc = pcpool.tile([128, 8], F32, tag="pc")
                nc.tensor.matmul(pc[:], ltri[:], tt[:], start=True, stop=True)

                # w = exp(within-row suffix + carry)
                wpre = wpool.tile([128, 128], F32, tag="wpre")
                nc.vector.tensor_add(
                    wpre[:].rearrange("q (i h) -> q i h", i=NT),
                    va[:, 0:128].rearrange("q (i h) -> q i h", i=NT),
                    pc[:].unsqueeze(1).broadcast_to((128, NT, 8)),
                )
                w = wpool.tile([128, 128], F32, tag="w")
                nc.scalar.activation(w[:], wpre[:], mybir.ActivationFunctionType.Exp)

                # ---- B load (contiguous) + decay weighting (broadcast over n) ----
                bt = bpool.tile([128, NT * 128], F32)
                nc.sync.dma_start(
                    out=bt[:].rearrange("q (i h n) -> q i h n", i=NT, h=NH),
                    in_=Bd[b].rearrange("(q i) h n -> q i h n", q=128),
                )
                bw = bwpool.tile([128, NT * 128], F32)
                nc.vector.tensor_mul(
                    bw[:].rearrange("q (ih n) -> q ih n", n=DN),
                    bt[:].rearrange("q (ih n) -> q ih n", n=DN),
                    w[:].unsqueeze(2).broadcast_to((128, 128, DN)),
                )

                # ---- X load (contiguous 4 MB, 32 KB runs) ----
                xt = xpool.tile([128, NT * 512], F32)
                nc.sync.dma_start(
                    out=xt[:].rearrange("q (i h p) -> q i h p", i=NT, h=NH),
                    in_=Xd[b].rearrange("(q i) h p -> q i h p", q=128),
                )

                # ---- main matmuls: stationary Bw tile, moving X tile ----
                pm = pmpool.tile([128, 512], F32, tag="pm")
                for i in range(NT):
                    nc.tensor.matmul(
                        pm[:],
                        bw[:, i * 128 : (i + 1) * 128],
                        xt[:, i * 512 : (i + 1) * 512],
                        start=(i == 0),
                        stop=(i == NT - 1),
                    )

                # ---- blockwise transpose -> (p, n) blocks, then DMA out ----
                sb = opool.tile([128, 512], F32, tag="sb")
                nc.vector.tensor_copy(sb[:], pm[:])
                tb = opool.tile([128, 512], F32, tag="tb")
                nc.vector.transpose(tb[:], sb[:])
                for j in range(4):
                    blk = tb[32 * j : 32 * j + 32, 128 * j : 128 * j + 128].rearrange(
                        "p (k c) -> p k c", k=4
                    )
                    nc.scalar.dma_start(
                        out=Od[b, 2 * j].rearrange("(k pp) n -> pp k n", k=2),
                        in_=blk[:, 0:2, 0:16],
                    )
                    nc.scalar.dma_start(
                        out=Od[b, 2 * j + 1].rearrange("(k pp) n -> pp k n", k=2),
                        in_=blk[:, 2:4, 16:32],
                    )

    nc.compile()
    _NC_CACHE = nc
    return nc


def run(inputs, trace=False, tmpdir=None, trace_kwargs=None):
    """Run the SPMD kernel on 8 cores.  Returns (output, BassKernelResults)."""
    X = np.asarray(inputs["X"], dtype=np.float32)
    A = np.asarray(inputs["A"], dtype=np.float32)
    B = np.asarray(inputs["B"], dtype=np.float32)
    assert X.shape == (NCORES * NB, T, NH, DP), X.shape

    nc = _build()
    in_maps = []
    for c in range(NCORES):
        s = slice(c * NB, (c + 1) * NB)
        in_maps.append(
            {
                "X": np.ascontiguousarray(X[s]),
                "A": np.ascontiguousarray(A[s]),
                "B": np.ascontiguousarray(B[s]),
            }
        )
    kw = {}
    if trace:
        kw.update(trace=True, tmpdir=tmpdir, trace_kwargs=trace_kwargs or {})
    res = run_bass_kernel_spmd(nc, in_maps, core_ids=list(range(NCORES)), **kw)
    out = np.concatenate([res.results[c]["O"] for c in range(NCORES)], axis=0)
    return out, res


def kernel(**inputs) -> np.ndarray:
    out, _ = run(inputs)
    return out
